# revision 1
# baseline (speedup 1.0000x reference)
"""Trainium2 Bass kernel for sliding-window causal self-attention (GQA + RoPE +
RMS-norm QK + value-embedding gating).

Sharding: 8 cores = 2 (batch) x 4 (KV groups).  Each core handles one batch
element and one KV head (= 4 query heads), computes a partial output through
the row-slice of Wproj for its heads; the host sums the 4 partials per batch.
"""

import sys
import os

for _p in ("/root/.axon_site", "/root/.axon_site/_ro/trn_rl_repo",
           "/root/.axon_site/_ro/pypackages", "/opt/trn_rl_repo"):
    if os.path.isdir(_p) and _p not in sys.path:
        sys.path.append(_p)

import numpy as np
import ml_dtypes
from contextlib import ExitStack

import concourse.bass as bass
import concourse.tile as tile
from concourse import bacc, mybir
from concourse.bass_utils import run_bass_kernel_spmd

BF16 = ml_dtypes.bfloat16
N_HEAD, N_KV, HEAD_DIM, WINDOW, N_EMBD = 16, 4, 64, 512, 1024
B, T = 2, 2048
NCORES = 8
TCH = 512               # token chunk for the projection phase
NCH = T // TCH          # 4
NTT = T // 128          # 16 t-tiles
HPK = N_HEAD // N_KV    # 4 query heads per core

F32 = mybir.dt.float32
BF = mybir.dt.bfloat16
AF = mybir.ActivationFunctionType
OP = mybir.AluOpType

_cache = {}


def _build(debug_taps=False):
    nc = bacc.Bacc("TRN2", target_bir_lowering=False, debug=False,
                   num_devices=NCORES)

    xt_d = nc.dram_tensor("xt", [8, 128, T], BF, kind="ExternalInput")
    wq_d = nc.dram_tensor("wq", [8, 128, 256], BF, kind="ExternalInput")
    wkv_d = nc.dram_tensor("wkv", [8, 128, 128], BF, kind="ExternalInput")
    wg_d = nc.dram_tensor("wg", [32, 1], BF, kind="ExternalInput")
    wp_d = nc.dram_tensor("wp", [2, 128, 1024], BF, kind="ExternalInput")
    cs1_d = nc.dram_tensor("cs1", [128, T], F32, kind="ExternalInput")
    cs2_d = nc.dram_tensor("cs2", [128, T], F32, kind="ExternalInput")
    ve_d = nc.dram_tensor("ve2", [16, 128, 64], BF, kind="ExternalInput")
    msk_d = nc.dram_tensor("masks", [128, 256], BF, kind="ExternalInput")
    id_d = nc.dram_tensor("ident", [64, 64], BF, kind="ExternalInput")
    selq_d = nc.dram_tensor("selq", [128, 33], BF, kind="ExternalInput")
    on64_d = nc.dram_tensor("ones64", [64, 1], BF, kind="ExternalInput")
    on1x_d = nc.dram_tensor("ones1x64", [1, 64], BF, kind="ExternalInput")
    id1_d = nc.dram_tensor("id1", [1, 1], BF, kind="ExternalInput")
    out_d = nc.dram_tensor("out", [T, N_EMBD], F32, kind="ExternalOutput")
    if debug_taps:
        qdbg_d = nc.dram_tensor("qdbg", [4, 64, T], BF, kind="ExternalOutput")
        kdbg_d = nc.dram_tensor("kdbg", [64, T], BF, kind="ExternalOutput")
        vdbg_d = nc.dram_tensor("vdbg", [128, NTT * 65], BF, kind="ExternalOutput")
        rkdbg_d = nc.dram_tensor("rkdbg", [128, NTT], F32, kind="ExternalOutput")
        ytdbg_d = nc.dram_tensor("ytdbg", [2, 128, T], BF, kind="ExternalOutput")

    with tile.TileContext(nc) as tc, ExitStack() as ctx:
        pers = ctx.enter_context(tc.tile_pool(name="pers", bufs=1))
        work = ctx.enter_context(tc.tile_pool(name="work", bufs=2))
        ptw = ctx.enter_context(tc.tile_pool(name="ptw", bufs=6))
        outw = ctx.enter_context(tc.tile_pool(name="outw", bufs=3))
        # PSUM pools (8 banks total):
        pb512 = ctx.enter_context(tc.tile_pool(name="pb512", bufs=2, space="PSUM"))
        pb128 = ctx.enter_context(tc.tile_pool(name="pb128", bufs=2, space="PSUM"))
        pbyx = ctx.enter_context(tc.tile_pool(name="pbyx", bufs=2, space="PSUM"))
        pbsm = ctx.enter_context(tc.tile_pool(name="pbsm", bufs=2, space="PSUM"))

        # ---- persistent SBUF loads ----
        xt_sb = pers.tile([128, 8 * T], BF, tag="xt")
        for kt in range(8):
            nc.sync.dma_start(xt_sb[:, kt * T:(kt + 1) * T], xt_d[kt])
        wq_sb = pers.tile([128, 8 * 256], BF, tag="wq")
        for kt in range(8):
            nc.sync.dma_start(wq_sb[:, kt * 256:(kt + 1) * 256], wq_d[kt])
        wkv_sb = pers.tile([128, 8 * 128], BF, tag="wkv")
        for kt in range(8):
            nc.sync.dma_start(wkv_sb[:, kt * 128:(kt + 1) * 128], wkv_d[kt])
        wg_sb = pers.tile([32, 1], BF, tag="wg")
        nc.sync.dma_start(wg_sb[:], wg_d[:])
        wp_sb = pers.tile([128, 2 * 1024], BF, tag="wp")
        for p in range(2):
            nc.sync.dma_start(wp_sb[:, p * 1024:(p + 1) * 1024], wp_d[p])
        cs1_sb = pers.tile([128, T], F32, tag="cs1")
        nc.sync.dma_start(cs1_sb[:], cs1_d[:])
        cs2_sb = pers.tile([128, T], F32, tag="cs2")
        nc.sync.dma_start(cs2_sb[:], cs2_d[:])
        ve_sb = pers.tile([128, 16 * 64], BF, tag="ve")
        for j in range(16):
            nc.sync.dma_start(ve_sb[:, j * 64:(j + 1) * 64], ve_d[j])
        msk_sb = pers.tile([128, 256], BF, tag="msk")
        nc.sync.dma_start(msk_sb[:], msk_d[:])
        id_sb = pers.tile([64, 64], BF, tag="ident")
        nc.sync.dma_start(id_sb[:], id_d[:])
        selq_sb = pers.tile([128, 33], BF, tag="selq")
        nc.sync.dma_start(selq_sb[:], selq_d[:])
        on64_sb = pers.tile([64, 1], BF, tag="on64")
        nc.sync.dma_start(on64_sb[:], on64_d[:])
        on1x_sb = pers.tile([1, 64], BF, tag="on1x")
        nc.sync.dma_start(on1x_sb[:], on1x_d[:])
        id1_sb = pers.tile([1, 1], BF, tag="id1")
        nc.sync.dma_start(id1_sb[:], id1_d[:])

        # ---- persistent intermediates ----
        qt_sb = [pers.tile([64, T], BF, tag=f"qt{h}", name=f"qt{h}")
                 for h in range(4)]          # Q^T per head
        kt_sb = pers.tile([64, T], BF, tag="kt")     # K^T
        vn_sb = pers.tile([128, NTT * 65], BF, tag="vn")  # V natural + ones col
        yt_sb = [pers.tile([128, T], BF, tag=f"yt{p}", name=f"yt{p}")
                 for p in range(2)]          # y^T, heads stacked
        rk_sb = pers.tile([128, NTT], F32, tag="rk")  # K rms recip, natural

        nc.vector.memset(vn_sb[:], 1.0)      # ones columns (col 64 of each group)
        biasq_sb = pers.tile([128, 1], F32, tag="biasq")
        nc.vector.memset(biasq_sb[:], 64e-6)
        biask_sb = pers.tile([1, 1], F32, tag="biask")
        nc.vector.memset(biask_sb[:], 1e-6)

        # =========== Phase 1: projections + RoPE + RMS + V prep ===========
        for ch in range(NCH):
            c0 = ch * TCH
            csl = slice(c0, c0 + TCH)

            def qk_head_ops(ps, rows, dst, is_q):
                """RoPE + RMS for psum rows [rows, rows+64*n) -> dst slice."""
                n = (128 - rows) // 64 if not is_q else 2
                # rope: A = ps * cs1, B = ps * cs2 (row-aligned trig tiles)
                nr = 128 if is_q else 64
                # A rows hh:    x1*cos   | B rows hh:    x2*sin (shifted up)
                # A rows hh+32: x1*sin(dn)| B rows hh+32: x2*cos
                A = work.tile([128, TCH], F32, tag="ropeA", name="ropeA")
                Bt = work.tile([128, TCH], F32, tag="ropeB", name="ropeB")
                ro = work.tile([128, TCH], F32, tag="rope", name="rope")
                for hh in range(0, nr, 64):
                    h1 = slice(hh, hh + 32)
                    h2 = slice(hh + 32, hh + 64)
                    nc.vector.tensor_mul(A[h1], ps[h1], cs1_sb[h1, csl])
                    nc.vector.tensor_mul(Bt[h1], ps[h2], cs1_sb[h2, csl])
                    nc.vector.tensor_mul(A[h2], ps[h1], cs2_sb[h1, csl])
                    nc.vector.tensor_mul(Bt[h2], ps[h2], cs2_sb[h2, csl])
                    nc.vector.tensor_sub(ro[h1], A[h1], Bt[h1])
                    nc.vector.tensor_add(ro[h2], A[h2], Bt[h2])
                # rms: recip = 1/sqrt(ss*sc + bias); Q folds the 1/8 score scale
                sq = work.tile([128, TCH], BF, tag="sq", name="sq")
                nc.scalar.square(sq[:nr], ro[0:nr])
                if is_q:
                    ss = pb512.tile([33, TCH], F32, tag="b512", name="ssq")
                    nc.tensor.matmul(ss[:], selq_sb[:], sq[:], start=True,
                                     stop=True)
                    bcps = pbyx.tile([128, TCH], F32, tag="yx", name="bcps")
                    for i in range(2):
                        r = 32 * i
                        srt = work.tile([1, TCH], F32, tag=f"srt{i}",
                                        name=f"srt{i}")
                        nc.scalar.activation(srt[:], ss[r:r + 1], AF.Sqrt,
                                             bias=biasq_sb[r:r + 1], scale=1.0)
                        rcpf = work.tile([1, TCH], F32, tag=f"rcpf{i}",
                                         name=f"rcpf{i}")
                        nc.vector.reciprocal_approx_fast(rcpf[:], srt[:])
                        rcp = work.tile([1, TCH], BF, tag=f"rcp{i}",
                                        name=f"rcp{i}")
                        nc.scalar.copy(rcp[:], rcpf[:])
                        nc.tensor.matmul(bcps[64 * i:64 * i + 64], on1x_sb[:],
                                         rcp[:], start=True, stop=True)
                    for i in range(2):
                        nc.vector.tensor_mul(dst[i][:, csl],
                                             ro[64 * i:64 * i + 64],
                                             bcps[64 * i:64 * i + 64])
                else:
                    # K^T stays unnormalized; rms recip folded into exp scale
                    nc.vector.tensor_copy(dst, ro[0:nr])
                    ss = pb512.tile([33, TCH], F32, tag="b512", name="ssk")
                    nc.tensor.matmul(ss[0:1], on64_sb[:], sq[0:64], start=True,
                                     stop=True)
                    srt = work.tile([1, TCH], F32, tag="srt0", name="srtk")
                    nc.scalar.activation(srt[:], ss[0:1], AF.Sqrt,
                                         bias=biask_sb[:], scale=1.0 / 64)
                    rcpkf = work.tile([1, TCH], F32, tag="rcpf0",
                                      name="rcpkf")
                    nc.vector.reciprocal_approx_fast(rcpkf[:], srt[:])
                    rcpk = work.tile([1, TCH], BF, tag="rcp0", name="rcpk")
                    nc.scalar.copy(rcpk[:], rcpkf[:])
                    for j in range(4):
                        rkp = pbsm.tile([128, 1], BF, tag="sm", name="rkp")
                        nc.tensor.transpose(
                            rkp[:], rcpk[:, j * 128:(j + 1) * 128], id1_sb[:])
                        tt = ch * 4 + j
                        nc.scalar.copy(rk_sb[:, tt:tt + 1], rkp[:])

            # Q pairs
            for p in range(2):
                psq = pb512.tile([128, TCH], F32, tag="b512", name="psq")
                for kt in range(8):
                    nc.tensor.matmul(
                        psq[:], wq_sb[:, kt * 256 + p * 128: kt * 256 + (p + 1) * 128],
                        xt_sb[:, kt * T + c0: kt * T + c0 + TCH],
                        start=(kt == 0), stop=(kt == 7))
                qk_head_ops(psq, 0, qt_sb[2 * p:2 * p + 2], True)

            # K | V^T
            pskv = pb512.tile([128, TCH], F32, tag="b512", name="pskv")
            for kt in range(8):
                nc.tensor.matmul(
                    pskv[:], wkv_sb[:, kt * 128:(kt + 1) * 128],
                    xt_sb[:, kt * T + c0: kt * T + c0 + TCH],
                    start=(kt == 0), stop=(kt == 7))
            qk_head_ops(pskv, 0, kt_sb[:, csl], False)
            vt_bf = work.tile([64, TCH], BF, tag="vt", name="vt")
            nc.scalar.copy(vt_bf[:], pskv[64:128])

            # V natural (+ gate * ve) per t-tile
            for j in range(4):
                t0 = c0 + j * 128
                tt = ch * 4 + j
                vtp = pbsm.tile([128, 64], BF, tag="sm", name="vtp")
                nc.tensor.transpose(vtp[:], vt_bf[:, j * 128:(j + 1) * 128],
                                    id_sb[:])
                gps = pbsm.tile([128, 64], F32, tag="sm", name="gps")
                nc.tensor.matmul(gps[:, 0:1], xt_sb[0:32, t0:t0 + 128],
                                 wg_sb[:], start=True, stop=True)
                g_sb = outw.tile([128, 1], F32, tag="g", name="g")
                nc.scalar.activation(g_sb[:], gps[:, 0:1], AF.Sigmoid)
                nc.vector.scalar_tensor_tensor(
                    vn_sb[:, tt * 65: tt * 65 + 64],
                    ve_sb[:, tt * 64:(tt + 1) * 64], g_sb[:], vtp[:],
                    op0=OP.mult, op1=OP.add)

        # ====== Phase 2+3: attention per q-tile, then output projection ======
        for qt in range(NTT):
            lo = max(0, qt - 4)
            for h in range(HPK):
                p, hh = h // 2, (h % 2) * 64
                q_ap = qt_sb[h][:, qt * 128:(qt + 1) * 128]
                yext = pbyx.tile([65, 128], F32, tag="yx", name="yext")
                for kt in range(lo, qt + 1):
                    stp = pb128.tile([128, 128], F32, tag="st", name="stp")
                    nc.tensor.matmul(stp[:],
                                     kt_sb[:, kt * 128:(kt + 1) * 128], q_ap,
                                     start=True, stop=True)
                    pt = ptw.tile([128, 128], BF, tag="pt", name="pt")
                    nc.scalar.activation(pt[:], stp[:], AF.Exp,
                                         scale=rk_sb[:, kt:kt + 1])
                    if kt == qt:
                        nc.vector.tensor_mul(pt[:], pt[:], msk_sb[:, 0:128])
                    elif kt == qt - 4:
                        nc.vector.tensor_mul(pt[:], pt[:], msk_sb[:, 128:256])
                    nc.tensor.matmul(yext[:],
                                     vn_sb[:, kt * 65: kt * 65 + 65], pt[:],
                                     start=(kt == lo), stop=(kt == qt))
                rrf = outw.tile([1, 128], F32, tag="rrf", name="rrf")
                nc.vector.reciprocal(rrf[:], yext[64:65, :])
                rr = outw.tile([1, 128], BF, tag="rr", name="rr")
                nc.scalar.copy(rr[:], rrf[:])
                bcq = pbsm.tile([64, 128], F32, tag="sm", name="bcq")
                nc.tensor.matmul(bcq[:], on1x_sb[:], rr[:], start=True,
                                 stop=True)
                bca = outw.tile([64, 128], BF, tag="bca", name="bca")
                nc.scalar.copy(bca[:], bcq[:])
                nc.vector.tensor_mul(
                    yt_sb[p][hh:hh + 64, qt * 128:(qt + 1) * 128],
                    yext[0:64, :], bca[:])

            if debug_taps and qt == NTT - 1:
                for h in range(4):
                    nc.sync.dma_start(qdbg_d[h], qt_sb[h][:])
                nc.sync.dma_start(kdbg_d[:], kt_sb[:])
                nc.sync.dma_start(vdbg_d[:], vn_sb[:])
                nc.sync.dma_start(rkdbg_d[:], rk_sb[:])
            if debug_taps and qt == NTT - 1:
                for p in range(2):
                    nc.sync.dma_start(ytdbg_d[p], yt_sb[p][:])
            # output projection for this t-tile
            for cc in range(2):
                ops = pb512.tile([128, TCH], F32, tag="b512", name="ops")
                for p in range(2):
                    nc.tensor.matmul(
                        ops[:], yt_sb[p][:, qt * 128:(qt + 1) * 128],
                        wp_sb[:, p * 1024 + cc * 512: p * 1024 + cc * 512 + 512],
                        start=(p == 0), stop=(p == 1))
                o_sb = outw.tile([128, TCH], F32, tag="osb", name="osb")
                if cc == 0:
                    nc.scalar.copy(o_sb[:], ops[:])
                else:
                    nc.vector.tensor_copy(o_sb[:], ops[:])
                nc.sync.dma_start(
                    out_d[qt * 128:(qt + 1) * 128, cc * 512:(cc + 1) * 512],
                    o_sb[:])

    nc.compile()
    return nc


def _prep_inputs(x, ve, cos, sin, Wq, Wk, Wv, Wproj, Wgate):
    """Build the 8 per-core input maps (host-side sharding + layout prep)."""
    cosT = np.ascontiguousarray(cos.T).astype(np.float32)   # [32, T]
    sinT = np.ascontiguousarray(sin.T).astype(np.float32)
    cs1 = np.concatenate([cosT, sinT, cosT, sinT], 0)       # [128, T]
    cs2 = np.concatenate([sinT, cosT, sinT, cosT], 0)
    masks = np.concatenate([
        np.triu(np.ones((128, 128), np.float32)),           # causal (col>=row)
        np.tril(np.ones((128, 128), np.float32)),           # window (col<=row)
    ], 1).astype(BF16)
    ident = np.eye(64, dtype=BF16)
    selq = np.zeros((128, 33), np.float32)
    selq[0:64, 0] = 1.0
    selq[64:128, 32] = 1.0
    selq = selq.astype(BF16)
    ones64 = np.ones((64, 1), BF16)
    ones1x64 = np.ones((1, 64), BF16)
    id1 = np.ones((1, 1), BF16)

    xT = [np.ascontiguousarray(x[b].astype(BF16).T).reshape(8, 128, T)
          for b in range(B)]
    in_maps = []
    for c in range(NCORES):
        b, g = c // 4, c % 4
        wq_g = np.ascontiguousarray(
            Wq[:, g * 256:(g + 1) * 256]).astype(BF16).reshape(8, 128, 256)
        wkv_g = np.concatenate(
            [Wk[:, g * 64:(g + 1) * 64], Wv[:, g * 64:(g + 1) * 64]],
            1).astype(BF16).reshape(8, 128, 128)
        wg_g = np.ascontiguousarray(Wgate[:, g:g + 1]).astype(BF16)
        wp_g = np.ascontiguousarray(
            Wproj[g * 256:(g + 1) * 256, :]).astype(BF16).reshape(2, 128, 1024)
        ve_g = np.ascontiguousarray(
            2.0 * ve[b, :, g * 64:(g + 1) * 64]).astype(BF16).reshape(16, 128, 64)
        in_maps.append({
            "xt": xT[b], "wq": wq_g, "wkv": wkv_g, "wg": wg_g, "wp": wp_g,
            "cs1": cs1, "cs2": cs2, "ve2": ve_g, "masks": masks,
            "ident": ident, "selq": selq, "ones64": ones64,
            "ones1x64": ones1x64, "id1": id1,
        })
    return in_maps


def _run(inputs, trace=False, tmpdir=None):
    if "nc" not in _cache:
        _cache["nc"] = _build()
    nc = _cache["nc"]
    in_maps = _prep_inputs(**inputs)
    res = run_bass_kernel_spmd(nc, in_maps, list(range(NCORES)), trace=trace,
                               tmpdir=tmpdir)
    out = np.zeros((B, T, N_EMBD), np.float32)
    for c in range(NCORES):
        out[c // 4] += np.asarray(res.results[c]["out"], np.float32)
    return out, res


def kernel(**inputs):
    out, _ = _run(inputs)
    return out



# revision 15
# speedup vs baseline: 1.4401x; 1.4401x over previous
"""Trainium2 Bass kernel for sliding-window causal self-attention (GQA + RoPE +
RMS-norm QK + value-embedding gating).

Sharding: 8 cores = 2 (batch) x 4 (KV groups).  Each core handles one batch
element and one KV head (= 4 query heads), computes a partial output through
the row-slice of Wproj for its heads; the host sums the 4 partials per batch.

v2: head-batched phase 2 (one [128,512] score/exp/AV op per (qt,kt) covering
all 4 query heads), single ACT table set (rsqrt/sigmoid rewritten with ln/exp),
approx reciprocals, gpsimd mask muls, interleaved phase emission, bf16 output.
"""

import sys
import os

for _p in ("/root/.axon_site", "/root/.axon_site/_ro/trn_rl_repo",
           "/root/.axon_site/_ro/pypackages", "/opt/trn_rl_repo"):
    if os.path.isdir(_p) and _p not in sys.path:
        sys.path.append(_p)

import numpy as np
import ml_dtypes
from contextlib import ExitStack

import concourse.bass as bass
import concourse.tile as tile
from concourse import bacc, mybir
from concourse.bass_utils import run_bass_kernel_spmd

BF16 = ml_dtypes.bfloat16
N_HEAD, N_KV, HEAD_DIM, WINDOW, N_EMBD = 16, 4, 64, 512, 1024
B, T = 2, 2048
NCORES = 8
TCH = 512               # token chunk for the projection phase
NCH = T // TCH          # 4
NTT = T // 128          # 16 t-tiles
HPK = N_HEAD // N_KV    # 4 query heads per core

F32 = mybir.dt.float32
BF = mybir.dt.bfloat16
AF = mybir.ActivationFunctionType
OP = mybir.AluOpType

# engine choices (tunable)
MASK_ENGINE = "gpsimd"      # "gpsimd" | "vector"
BCA_GPSIMD_PB = False       # broadcast denom recip via gpsimd partition_broadcast
DEBUG_TAPS = False

_cache = {}


def _build():
    nc = bacc.Bacc("TRN2", target_bir_lowering=False, debug=False,
                   num_devices=NCORES)

    xt_d = nc.dram_tensor("xt", [8, 128, T], BF, kind="ExternalInput")
    wq_d = nc.dram_tensor("wq", [8, 128, 256], BF, kind="ExternalInput")
    wkv_d = nc.dram_tensor("wkv", [8, 128, 128], BF, kind="ExternalInput")
    wg_d = nc.dram_tensor("wg", [32, 1], BF, kind="ExternalInput")
    wp_d = nc.dram_tensor("wp", [2, 128, 1024], BF, kind="ExternalInput")
    cs1_d = nc.dram_tensor("cs1", [128, T], F32, kind="ExternalInput")
    cs2_d = nc.dram_tensor("cs2", [128, T], F32, kind="ExternalInput")
    ve_d = nc.dram_tensor("ve2", [16, 128, 64], BF, kind="ExternalInput")
    mska_d = nc.dram_tensor("mska", [128, 512], BF, kind="ExternalInput")
    mskb_d = nc.dram_tensor("mskb", [128, 512], BF, kind="ExternalInput")
    id_d = nc.dram_tensor("ident", [64, 64], BF, kind="ExternalInput")
    selq_d = nc.dram_tensor("selq", [128, 33], BF, kind="ExternalInput")
    on64_d = nc.dram_tensor("ones64", [64, 1], BF, kind="ExternalInput")
    on1x_d = nc.dram_tensor("ones1x64", [1, 64], BF, kind="ExternalInput")
    id1_d = nc.dram_tensor("id1", [1, 1], BF, kind="ExternalInput")
    out_d = nc.dram_tensor("out", [T, N_EMBD], BF, kind="ExternalOutput")
    if DEBUG_TAPS:
        q4dbg_d = nc.dram_tensor("q4dbg", [64, NTT * 512], BF, kind="ExternalOutput")
        ktdbg_d = nc.dram_tensor("ktdbg", [64, T], BF, kind="ExternalOutput")
        vndbg_d = nc.dram_tensor("vndbg", [128, NTT * 65], BF, kind="ExternalOutput")
        rkdbg_d = nc.dram_tensor("rkdbg", [128, 2 * NTT], F32, kind="ExternalOutput")
        ytdbg_d = nc.dram_tensor("ytdbg", [2, 128, T], BF, kind="ExternalOutput")
        rrdbg_d = nc.dram_tensor("rrdbg", [NTT, 512], F32, kind="ExternalOutput")
        dendbg_d = nc.dram_tensor("dendbg", [NTT, 512], F32, kind="ExternalOutput")
        stpdbg_d = nc.dram_tensor("stpdbg", [128, 512], F32, kind="ExternalOutput")
        ptdbg_d = nc.dram_tensor("ptdbg", [128, 512], BF, kind="ExternalOutput")

    with tile.TileContext(nc) as tc, ExitStack() as ctx:
        pers = ctx.enter_context(tc.tile_pool(name="pers", bufs=1))
        work = ctx.enter_context(tc.tile_pool(name="work", bufs=2))
        ptw = ctx.enter_context(tc.tile_pool(name="ptw", bufs=4))
        outw = ctx.enter_context(tc.tile_pool(name="outw", bufs=2))
        # PSUM pools: 2+2+2+2 banks = 16KB
        pbig = ctx.enter_context(tc.tile_pool(name="pbig", bufs=2, space="PSUM"))
        pacc = ctx.enter_context(tc.tile_pool(name="pacc", bufs=2, space="PSUM"))
        psm = ctx.enter_context(tc.tile_pool(name="psm", bufs=2, space="PSUM"))
        pprj = ctx.enter_context(tc.tile_pool(name="pprj", bufs=1, space="PSUM"))

        # ---- persistent SBUF tiles ----
        xt_sb = pers.tile([128, 8 * T], BF, tag="xt")
        wq_sb = pers.tile([128, 8 * 256], BF, tag="wq")
        wkv_sb = pers.tile([128, 8 * 128], BF, tag="wkv")
        wg_sb = pers.tile([32, 1], BF, tag="wg")
        wp_sb = pers.tile([128, 2 * 1024], BF, tag="wp")
        cs1_sb = pers.tile([128, T], F32, tag="cs1")
        cs2_sb = pers.tile([128, T], F32, tag="cs2")
        ve_sb = pers.tile([128, 16 * 64], BF, tag="ve")
        mska_sb = pers.tile([128, 512], BF, tag="mska")
        mskb_sb = pers.tile([128, 512], BF, tag="mskb")
        id_sb = pers.tile([64, 64], BF, tag="ident")
        selq_sb = pers.tile([128, 33], BF, tag="selq")
        on64_sb = pers.tile([64, 1], BF, tag="on64")
        on1x_sb = pers.tile([1, 64], BF, tag="on1x")
        id1_sb = pers.tile([1, 1], BF, tag="id1")

        # weights + small constants first (needed by every chunk)
        for kt in range(8):
            nc.sync.dma_start(wq_sb[:, kt * 256:(kt + 1) * 256], wq_d[kt])
        for kt in range(8):
            nc.sync.dma_start(wkv_sb[:, kt * 128:(kt + 1) * 128], wkv_d[kt])
        nc.sync.dma_start(wg_sb[:], wg_d[:])
        for p in range(2):
            nc.sync.dma_start(wp_sb[:, p * 1024:(p + 1) * 1024], wp_d[p])
        nc.sync.dma_start(mska_sb[:], mska_d[:])
        nc.sync.dma_start(mskb_sb[:], mskb_d[:])
        nc.sync.dma_start(id_sb[:], id_d[:])
        nc.sync.dma_start(selq_sb[:], selq_d[:])
        nc.sync.dma_start(on64_sb[:], on64_d[:])
        nc.sync.dma_start(on1x_sb[:], on1x_d[:])
        nc.sync.dma_start(id1_sb[:], id1_d[:])

        # ---- persistent intermediates ----
        q4_sb = pers.tile([64, NTT * 512], BF, tag="q4")    # Q^T, head-interleaved
        kt_sb = pers.tile([64, T], BF, tag="kt")            # K^T
        vn_sb = pers.tile([128, NTT * 65], BF, tag="vn")    # V natural + ones col
        yt_sb = [pers.tile([128, T], BF, tag=f"yt{p}", name=f"yt{p}")
                 for p in range(2)]
        rk_sb = pers.tile([128, 2 * NTT], F32, tag="rk")    # K rms recip (even cols)

        nc.vector.memset(vn_sb[:], 1.0)
        biasq_sb = pers.tile([128, 1], F32, tag="biasq")
        nc.vector.memset(biasq_sb[:], 64e-6)
        biask_sb = pers.tile([128, 1], F32, tag="biask")
        nc.vector.memset(biask_sb[:], 1e-6)
        ones128_sb = pers.tile([128, 64], BF, tag="ones128")
        nc.vector.memset(ones128_sb[:], 1.0)

        def emit_dma(ch):
            c0 = ch * TCH
            for kt in range(8):
                nc.sync.dma_start(xt_sb[:, kt * T + c0: kt * T + c0 + TCH],
                                  xt_d[kt][:, c0:c0 + TCH])
            nc.sync.dma_start(cs1_sb[:, c0:c0 + TCH], cs1_d[:, c0:c0 + TCH])
            nc.sync.dma_start(cs2_sb[:, c0:c0 + TCH], cs2_d[:, c0:c0 + TCH])
            for j in range(4):
                tt = ch * 4 + j
                nc.sync.dma_start(ve_sb[:, tt * 64:(tt + 1) * 64], ve_d[tt])

        def rope(ps, nrow, name, csl):
            """RoPE on psum rows [0, nrow); returns f32 SBUF tile ro."""
            A = work.tile([128, TCH], F32, tag="ropeA", name=f"A{name}")
            Bt = work.tile([128, TCH], F32, tag="ropeB", name=f"B{name}")
            ro = work.tile([128, TCH], F32, tag="rope", name=f"ro{name}")
            for r in range(0, nrow, 64):
                h1 = slice(r, r + 32)
                h2 = slice(r + 32, r + 64)
                nc.vector.tensor_mul(A[h1], ps[h1], cs1_sb[h1, csl])
                nc.vector.tensor_mul(Bt[h1], ps[h2], cs1_sb[h2, csl])
                nc.vector.tensor_mul(A[h2], ps[h1], cs2_sb[h1, csl])
                nc.vector.tensor_mul(Bt[h2], ps[h2], cs2_sb[h2, csl])
                nc.vector.tensor_sub(ro[h1], A[h1], Bt[h1])
                nc.vector.tensor_add(ro[h2], A[h2], Bt[h2])
            return ro

        def emit_ph1_a(ch):
            """QKV projection matmuls (PE-dense)."""
            c0 = ch * TCH
            psq = [pbig.tile([128, TCH], F32, tag="mm", name=f"psq{p}_{ch}")
                   for p in range(2)]
            for p in range(2):
                for kt in range(8):
                    nc.tensor.matmul(
                        psq[p][:],
                        wq_sb[:, kt * 256 + p * 128: kt * 256 + (p + 1) * 128],
                        xt_sb[:, kt * T + c0: kt * T + c0 + TCH],
                        start=(kt == 0), stop=(kt == 7))
            pskv = pbig.tile([128, TCH], F32, tag="mm", name=f"pskv{ch}")
            for kt in range(8):
                nc.tensor.matmul(
                    pskv[:], wkv_sb[:, kt * 128:(kt + 1) * 128],
                    xt_sb[:, kt * T + c0: kt * T + c0 + TCH],
                    start=(kt == 0), stop=(kt == 7))
            return psq, pskv

        def emit_ph1_b(ch, psq):
            """Q: rope + rms (ln/exp rsqrt) -> q4 interleaved."""
            c0 = ch * TCH
            csl = slice(c0, c0 + TCH)
            ros = []
            sqs = []
            for p in range(2):
                ro = rope(psq[p], 128, f"q{p}_{ch}", csl)
                sq = work.tile([128, TCH], BF, tag="sq", name=f"sq{p}_{ch}")
                nc.vector.tensor_mul(sq[:], ro[:], ro[:])
                ros.append(ro)
                sqs.append(sq)
            ss = pacc.tile([97, TCH], F32, tag="acc", name=f"ss{ch}")
            nc.tensor.matmul(ss[0:33], selq_sb[:], sqs[0][:],
                             start=True, stop=True)
            nc.tensor.matmul(ss[64:97], selq_sb[:], sqs[1][:],
                             start=True, stop=True)
            lnt = work.tile([97, TCH], F32, tag="lnt", name=f"lnt{ch}")
            nc.scalar.activation(lnt[:], ss[:], AF.Ln, bias=biasq_sb[0:97],
                                 scale=1.0)
            rq = work.tile([97, TCH], BF, tag="rq", name=f"rq{ch}")
            nc.scalar.activation(rq[:], lnt[:], AF.Exp, scale=-0.5)
            for p in range(2):
                bcps = psm.tile([128, TCH], F32, tag="sm", name=f"bcps{p}_{ch}")
                r0, r1 = 64 * p, 64 * p + 32
                nc.tensor.matmul(bcps[0:64], ones128_sb[r0:r0 + 1, :],
                                 rq[r0:r0 + 1, :], start=True, stop=True,
                                 tile_position=(r0, 0))
                nc.tensor.matmul(bcps[64:128], ones128_sb[r1:r1 + 1, :],
                                 rq[r1:r1 + 1, :], start=True, stop=True,
                                 tile_position=(r1, 64))
                ro = ros[p]
                for i in range(2):
                    for j in range(4):
                        qt = ch * 4 + j
                        h = 2 * p + i
                        nc.vector.tensor_mul(
                            q4_sb[:, qt * 512 + h * 128: qt * 512 + (h + 1) * 128],
                            ro[64 * i: 64 * i + 64, j * 128:(j + 1) * 128],
                            bcps[64 * i: 64 * i + 64, j * 128:(j + 1) * 128])

        def emit_ph1_c(ch, pskv):
            """K: rope + rms->rk; V: transpose + gated ve add -> vn."""
            c0 = ch * TCH
            csl = slice(c0, c0 + TCH)
            ro = rope(pskv, 64, f"k{ch}", csl)
            nc.vector.tensor_copy(kt_sb[:, c0:c0 + TCH], ro[0:64])
            sqk = work.tile([64, TCH], BF, tag="sqk", name=f"sqk{ch}")
            nc.vector.tensor_mul(sqk[:], ro[0:64], ro[0:64])
            ssk = psm.tile([1, TCH], F32, tag="sm", name=f"ssk{ch}")
            nc.tensor.matmul(ssk[:], on64_sb[:], sqk[:], start=True, stop=True)
            sskb = work.tile([1, TCH], BF, tag="sskb", name=f"sskb{ch}")
            nc.vector.tensor_copy(sskb[:], ssk[:])
            rkT = psm.tile([128, 8], BF, tag="sm", name=f"rkT{ch}")
            for j in range(4):
                nc.tensor.transpose(rkT[:, 2 * j:2 * j + 1],
                                    sskb[:, j * 128:(j + 1) * 128], id1_sb[:])
            lnk = work.tile([128, 8], F32, tag="lnk", name=f"lnk{ch}")
            nc.scalar.activation(lnk[:], rkT[:], AF.Ln, bias=biask_sb[:],
                                 scale=1.0 / 64)
            nc.scalar.activation(rk_sb[:, ch * 8: ch * 8 + 8], lnk[:], AF.Exp,
                                 scale=-0.5)
            # gate: g = 2*sigmoid(z) = 2/(1+exp(-z)), z = x[:, :32] @ wg
            gps = psm.tile([1, TCH], F32, tag="sm", name=f"gps{ch}")
            nc.tensor.matmul(gps[:], wg_sb[:],
                             xt_sb[0:32, c0:c0 + TCH], start=True, stop=True)
            gu = work.tile([1, TCH], F32, tag="gu", name=f"gu{ch}")
            nc.scalar.activation(gu[:], gps[:], AF.Exp, scale=-1.0)
            nc.vector.tensor_scalar_add(gu[:], gu[:], 1.0)
            gr = work.tile([1, TCH], F32, tag="gr", name=f"gr{ch}")
            nc.vector.reciprocal_approx_fast(gr[:], gu[:])
            grb = work.tile([1, TCH], BF, tag="grb", name=f"grb{ch}")
            nc.vector.tensor_copy(grb[:], gr[:])
            gT = psm.tile([128, 8], BF, tag="sm", name=f"gT{ch}")
            for j in range(4):
                nc.tensor.transpose(gT[:, 2 * j:2 * j + 1],
                                    grb[:, j * 128:(j + 1) * 128], id1_sb[:])
            g_sb = work.tile([128, 8], F32, tag="gsb", name=f"gsb{ch}")
            nc.vector.tensor_copy(g_sb[:], gT[:])
            # V natural + gate * ve
            vt_bf = work.tile([64, TCH], BF, tag="vt", name=f"vt{ch}")
            nc.vector.tensor_copy(vt_bf[:], pskv[64:128])
            for j in range(4):
                tt = ch * 4 + j
                vtp = psm.tile([128, 64], BF, tag="sm", name=f"vtp{tt}")
                nc.tensor.transpose(vtp[:], vt_bf[:, j * 128:(j + 1) * 128],
                                    id_sb[:])
                nc.vector.scalar_tensor_tensor(
                    vn_sb[:, tt * 65: tt * 65 + 64],
                    ve_sb[:, tt * 64:(tt + 1) * 64], g_sb[:, 2 * j:2 * j + 1], vtp[:],
                    op0=OP.mult, op1=OP.add)

        def emit_scores(qt):
            """scores + exp + mask + AV for one q-tile (4 heads wide)."""
            lo = max(0, qt - 4)
            yext = pacc.tile([97, 512], F32, tag="acc", name=f"yext{qt}")
            for kt in range(lo, qt + 1):
                stp = pbig.tile([128, 512], F32, tag="mm", name=f"stp{qt}_{kt}")
                nc.tensor.matmul(stp[:], kt_sb[:, kt * 128:(kt + 1) * 128],
                                 q4_sb[:, qt * 512:(qt + 1) * 512],
                                 start=True, stop=True)
                pt = ptw.tile([128, 512], BF, tag="pt", name=f"pt{qt}_{kt}")
                nc.scalar.activation(pt[:], stp[:], AF.Exp,
                                     scale=rk_sb[:, 2 * kt:2 * kt + 1])
                eng = nc.gpsimd if MASK_ENGINE == "gpsimd" else nc.vector
                if kt == qt:
                    eng.tensor_mul(pt[:], pt[:], mska_sb[:])
                elif kt == qt - 4:
                    eng.tensor_mul(pt[:], pt[:], mskb_sb[:])
                if DEBUG_TAPS and qt == 0:
                    stpc = work.tile([128, 512], F32, tag="stpc", name="stpc")
                    nc.vector.tensor_copy(stpc[:], stp[:])
                    nc.sync.dma_start(stpdbg_d[:], stpc[:])
                    nc.sync.dma_start(ptdbg_d[:], pt[:])
                nc.tensor.matmul(yext[0:65], vn_sb[:, kt * 65: kt * 65 + 65],
                                 pt[:], start=(kt == lo), stop=(kt == qt))
            return yext

        def emit_post(qt, yext):
            """denominator recip + broadcast + yt writes."""
            den = outw.tile([1, 512], F32, tag="den", name=f"den{qt}")
            nc.vector.tensor_copy(den[:], yext[64:65, :])
            rr = outw.tile([1, 512], F32, tag="rr", name=f"rr{qt}")
            nc.vector.reciprocal_approx_fast(rr[:], den[:])
            if DEBUG_TAPS:
                nc.sync.dma_start(rrdbg_d[qt:qt + 1, :], rr[:])
                denc = outw.tile([1, 512], F32, tag="denc", name=f"denc{qt}")
                nc.vector.tensor_copy(denc[:], yext[64:65, :])
                nc.sync.dma_start(dendbg_d[qt:qt + 1, :], denc[:])
            rrb = outw.tile([1, 512], BF, tag="rrb", name=f"rrb{qt}")
            nc.vector.tensor_copy(rrb[:], rr[:])
            if BCA_GPSIMD_PB:
                bca = outw.tile([64, 512], BF, tag="bca", name=f"bca{qt}")
                nc.gpsimd.partition_broadcast(bca[:], rrb[:])
                bsrc = bca
            else:
                bcq = psm.tile([64, 512], F32, tag="sm", name=f"bcq{qt}")
                nc.tensor.matmul(bcq[:], on1x_sb[:], rrb[:], start=True,
                                 stop=True)
                bca = outw.tile([64, 512], BF, tag="bca", name=f"bca{qt}")
                nc.vector.tensor_copy(bca[:], bcq[:])
                bsrc = bca
            for h in range(HPK):
                p, hh = h // 2, (h % 2) * 64
                nc.vector.tensor_mul(
                    yt_sb[p][hh:hh + 64, qt * 128:(qt + 1) * 128],
                    yext[0:64, h * 128:(h + 1) * 128],
                    bsrc[:, h * 128:(h + 1) * 128])

        def emit_proj(qt):
            ops = pprj.tile([128, 1024], F32, tag="prj", name=f"ops{qt}")
            for cc in range(2):
                for p in range(2):
                    nc.tensor.matmul(
                        ops[:, cc * 512:(cc + 1) * 512],
                        yt_sb[p][:, qt * 128:(qt + 1) * 128],
                        wp_sb[:, p * 1024 + cc * 512: p * 1024 + cc * 512 + 512],
                        start=(p == 0), stop=(p == 1))
            o_sb = outw.tile([128, 1024], BF, tag="osb", name=f"osb{qt}")
            nc.scalar.copy(o_sb[:, 0:512], ops[:, 0:512])
            nc.vector.tensor_copy(o_sb[:, 512:1024], ops[:, 512:1024])
            nc.sync.dma_start(out_d[qt * 128:(qt + 1) * 128, :], o_sb[:])

        # ---------------- emission schedule ----------------
        emit_dma(0)
        psq, pskv = emit_ph1_a(0)
        emit_ph1_b(0, psq)
        emit_ph1_c(0, pskv)
        for blk in range(4):
            if blk < 3:
                emit_dma(blk + 1)
                psq, pskv = emit_ph1_a(blk + 1)
                emit_ph1_b(blk + 1, psq)
                emit_ph1_c(blk + 1, pskv)
            for j in range(4):
                qt = blk * 4 + j
                yext = emit_scores(qt)
                emit_post(qt, yext)
                if qt > 0:
                    emit_proj(qt - 1)
        emit_proj(NTT - 1)
        if DEBUG_TAPS:
            nc.sync.dma_start(q4dbg_d[:], q4_sb[:])
            nc.sync.dma_start(ktdbg_d[:], kt_sb[:])
            nc.sync.dma_start(vndbg_d[:], vn_sb[:])
            nc.sync.dma_start(rkdbg_d[:], rk_sb[:])
            for p in range(2):
                nc.sync.dma_start(ytdbg_d[p], yt_sb[p][:])

    nc.compile()
    return nc


def _prep_inputs(x, ve, cos, sin, Wq, Wk, Wv, Wproj, Wgate):
    """Build the 8 per-core input maps (host-side sharding + layout prep)."""
    cosT = np.ascontiguousarray(cos.T).astype(np.float32)   # [32, T]
    sinT = np.ascontiguousarray(sin.T).astype(np.float32)
    cs1 = np.concatenate([cosT, sinT, cosT, sinT], 0)       # [128, T]
    cs2 = np.concatenate([sinT, cosT, sinT, cosT], 0)
    mska = np.tile(np.triu(np.ones((128, 128), np.float32)), (1, 4)).astype(BF16)
    mskb = np.tile(np.tril(np.ones((128, 128), np.float32)), (1, 4)).astype(BF16)
    ident = np.eye(64, dtype=BF16)
    selq = np.zeros((128, 33), np.float32)
    selq[0:64, 0] = 1.0
    selq[64:128, 32] = 1.0
    selq = selq.astype(BF16)
    ones64 = np.ones((64, 1), BF16)
    ones1x64 = np.ones((1, 64), BF16)
    id1 = np.ones((1, 1), BF16)

    xT = [np.ascontiguousarray(x[b].astype(BF16).T).reshape(8, 128, T)
          for b in range(B)]
    in_maps = []
    for c in range(NCORES):
        b, g = c // 4, c % 4
        wq_g = np.ascontiguousarray(
            Wq[:, g * 256:(g + 1) * 256]).astype(BF16).reshape(8, 128, 256)
        wkv_g = np.concatenate(
            [Wk[:, g * 64:(g + 1) * 64], Wv[:, g * 64:(g + 1) * 64]],
            1).astype(BF16).reshape(8, 128, 128)
        wg_g = np.ascontiguousarray(Wgate[:, g:g + 1]).astype(BF16)
        wp_g = np.ascontiguousarray(
            Wproj[g * 256:(g + 1) * 256, :]).astype(BF16).reshape(2, 128, 1024)
        ve_g = np.ascontiguousarray(
            2.0 * ve[b, :, g * 64:(g + 1) * 64]).astype(BF16).reshape(16, 128, 64)
        in_maps.append({
            "xt": xT[b], "wq": wq_g, "wkv": wkv_g, "wg": wg_g, "wp": wp_g,
            "cs1": cs1, "cs2": cs2, "ve2": ve_g, "mska": mska, "mskb": mskb,
            "ident": ident, "selq": selq, "ones64": ones64,
            "ones1x64": ones1x64, "id1": id1,
        })
    return in_maps


def _run(inputs, trace=False, tmpdir=None):
    if "nc" not in _cache:
        _cache["nc"] = _build()
    nc = _cache["nc"]
    in_maps = _prep_inputs(**inputs)
    res = run_bass_kernel_spmd(nc, in_maps, list(range(NCORES)), trace=trace,
                               tmpdir=tmpdir)
    out = np.zeros((B, T, N_EMBD), np.float32)
    for c in range(NCORES):
        out[c // 4] += np.asarray(res.results[c]["out"], np.float32)
    return out, res


def kernel(**inputs):
    out, _ = _run(inputs)
    return out


# revision 18
# speedup vs baseline: 1.5092x; 1.0480x over previous
"""Trainium2 Bass kernel for sliding-window causal self-attention (GQA + RoPE +
RMS-norm QK + value-embedding gating).

Sharding: 8 cores = 2 (batch) x 4 (KV groups).  Each core handles one batch
element and one KV head (= 4 query heads), computes a partial output through
the row-slice of Wproj for its heads; the host sums the 4 partials per batch.

v2: head-batched phase 2 (one [128,512] score/exp/AV op per (qt,kt) covering
all 4 query heads), single ACT table set (rsqrt/sigmoid rewritten with ln/exp),
approx reciprocals, gpsimd mask muls, interleaved phase emission, bf16 output.
"""

import sys
import os

for _p in ("/root/.axon_site", "/root/.axon_site/_ro/trn_rl_repo",
           "/root/.axon_site/_ro/pypackages", "/opt/trn_rl_repo"):
    if os.path.isdir(_p) and _p not in sys.path:
        sys.path.append(_p)

import numpy as np
import ml_dtypes
from contextlib import ExitStack

import concourse.bass as bass
import concourse.tile as tile
from concourse import bacc, mybir
from concourse.bass_utils import run_bass_kernel_spmd

BF16 = ml_dtypes.bfloat16
N_HEAD, N_KV, HEAD_DIM, WINDOW, N_EMBD = 16, 4, 64, 512, 1024
B, T = 2, 2048
NCORES = 8
TCH = 512               # token chunk for the projection phase
NCH = T // TCH          # 4
NTT = T // 128          # 16 t-tiles
HPK = N_HEAD // N_KV    # 4 query heads per core

F32 = mybir.dt.float32
BF = mybir.dt.bfloat16
AF = mybir.ActivationFunctionType
OP = mybir.AluOpType

# engine choices (tunable)
MASK_ENGINE = "gpsimd"      # "gpsimd" | "vector"
BCA_GPSIMD_PB = False       # broadcast denom recip via gpsimd partition_broadcast
DEBUG_TAPS = False

_cache = {}


def _patch_act_tables():
    """Restrict the ACT table-set chooser to natural_log_exp_and_others so
    Exp and Ln share one table set (avoids per-chunk ACT_TABLE_LOAD swaps)."""
    import concourse.hw_specs as hw_specs
    import concourse.bacc as bacc_mod
    if getattr(hw_specs, "_act_tables_patched", False):
        return
    orig = hw_specs.get_activation_tables
    def patched(arch):
        tabs = dict(orig(arch))
        keep = "natural_log_exp_and_others"
        return {name: (funcs if name == keep else frozenset())
                for name, funcs in tabs.items()}
    hw_specs._act_tables_patched = True
    hw_specs.get_activation_tables = patched
    bacc_mod.get_activation_tables = patched


def _build():
    _patch_act_tables()
    nc = bacc.Bacc("TRN2", target_bir_lowering=False, debug=False,
                   num_devices=NCORES)

    xt_d = nc.dram_tensor("xt", [8, 128, T], BF, kind="ExternalInput")
    wq_d = nc.dram_tensor("wq", [8, 128, 256], BF, kind="ExternalInput")
    wkv_d = nc.dram_tensor("wkv", [8, 128, 128], BF, kind="ExternalInput")
    wg_d = nc.dram_tensor("wg", [32, 1], BF, kind="ExternalInput")
    wp_d = nc.dram_tensor("wp", [2, 128, 1024], BF, kind="ExternalInput")
    cs1_d = nc.dram_tensor("cs1", [128, T], F32, kind="ExternalInput")
    cs2_d = nc.dram_tensor("cs2", [128, T], F32, kind="ExternalInput")
    ve_d = nc.dram_tensor("ve2", [16, 128, 64], BF, kind="ExternalInput")
    mska_d = nc.dram_tensor("mska", [128, 512], BF, kind="ExternalInput")
    mskb_d = nc.dram_tensor("mskb", [128, 512], BF, kind="ExternalInput")
    id_d = nc.dram_tensor("ident", [64, 64], BF, kind="ExternalInput")
    selq_d = nc.dram_tensor("selq", [128, 33], BF, kind="ExternalInput")
    on64_d = nc.dram_tensor("ones64", [64, 1], BF, kind="ExternalInput")
    on1x_d = nc.dram_tensor("ones1x64", [1, 64], BF, kind="ExternalInput")
    id1_d = nc.dram_tensor("id1", [1, 1], BF, kind="ExternalInput")
    out_d = nc.dram_tensor("out", [T, N_EMBD], BF, kind="ExternalOutput")
    if DEBUG_TAPS:
        q4dbg_d = nc.dram_tensor("q4dbg", [64, 4 * T], BF, kind="ExternalOutput")
        ktdbg_d = nc.dram_tensor("ktdbg", [64, T], BF, kind="ExternalOutput")
        vndbg_d = nc.dram_tensor("vndbg", [128, NTT * 65], BF, kind="ExternalOutput")
        rkdbg_d = nc.dram_tensor("rkdbg", [128, 2 * NTT], F32, kind="ExternalOutput")
        ytdbg_d = nc.dram_tensor("ytdbg", [2, 128, T], BF, kind="ExternalOutput")
        rrdbg_d = nc.dram_tensor("rrdbg", [NTT, 512], F32, kind="ExternalOutput")
        dendbg_d = nc.dram_tensor("dendbg", [NTT, 512], F32, kind="ExternalOutput")
        stpdbg_d = nc.dram_tensor("stpdbg", [128, 512], F32, kind="ExternalOutput")
        ptdbg_d = nc.dram_tensor("ptdbg", [128, 512], BF, kind="ExternalOutput")

    with tile.TileContext(nc) as tc, ExitStack() as ctx:
        pers = ctx.enter_context(tc.tile_pool(name="pers", bufs=1))
        work = ctx.enter_context(tc.tile_pool(name="work", bufs=2))
        ptw = ctx.enter_context(tc.tile_pool(name="ptw", bufs=4))
        outw = ctx.enter_context(tc.tile_pool(name="outw", bufs=2))
        # PSUM pools: 2+2+2+2 banks = 16KB
        pbig = ctx.enter_context(tc.tile_pool(name="pbig", bufs=2, space="PSUM"))
        pacc = ctx.enter_context(tc.tile_pool(name="pacc", bufs=2, space="PSUM"))
        psm = ctx.enter_context(tc.tile_pool(name="psm", bufs=2, space="PSUM"))
        pprj = ctx.enter_context(tc.tile_pool(name="pprj", bufs=1, space="PSUM"))

        # ---- persistent SBUF tiles ----
        xt_sb = pers.tile([128, 8 * T], BF, tag="xt")
        wq_sb = pers.tile([128, 8 * 256], BF, tag="wq")
        wkv_sb = pers.tile([128, 8 * 128], BF, tag="wkv")
        wg_sb = pers.tile([32, 1], BF, tag="wg")
        wp_sb = pers.tile([128, 2 * 1024], BF, tag="wp")
        cs1_sb = pers.tile([128, T], F32, tag="cs1")
        cs2_sb = pers.tile([128, T], F32, tag="cs2")
        ve_sb = pers.tile([128, 16 * 64], BF, tag="ve")
        mska_sb = pers.tile([128, 512], BF, tag="mska")
        mskb_sb = pers.tile([128, 512], BF, tag="mskb")
        id_sb = pers.tile([64, 64], BF, tag="ident")
        selq_sb = pers.tile([128, 33], BF, tag="selq")
        on64_sb = pers.tile([64, 1], BF, tag="on64")
        on1x_sb = pers.tile([1, 64], BF, tag="on1x")
        id1_sb = pers.tile([1, 1], BF, tag="id1")

        # weights + small constants first (needed by every chunk)
        for kt in range(8):
            nc.sync.dma_start(wq_sb[:, kt * 256:(kt + 1) * 256], wq_d[kt])
        for kt in range(8):
            nc.sync.dma_start(wkv_sb[:, kt * 128:(kt + 1) * 128], wkv_d[kt])
        nc.sync.dma_start(wg_sb[:], wg_d[:])
        for p in range(2):
            nc.sync.dma_start(wp_sb[:, p * 1024:(p + 1) * 1024], wp_d[p])
        nc.sync.dma_start(mska_sb[:], mska_d[:])
        nc.sync.dma_start(mskb_sb[:], mskb_d[:])
        nc.sync.dma_start(id_sb[:], id_d[:])
        nc.sync.dma_start(selq_sb[:], selq_d[:])
        nc.sync.dma_start(on64_sb[:], on64_d[:])
        nc.sync.dma_start(on1x_sb[:], on1x_d[:])
        nc.sync.dma_start(id1_sb[:], id1_d[:])

        # ---- persistent intermediates ----
        qn_sb = pers.tile([64, 4 * T], BF, tag="qn")   # Q^T head-major (0,2,1,3)
        kt_sb = pers.tile([64, T], BF, tag="kt")            # K^T
        vn_sb = pers.tile([128, NTT * 65], BF, tag="vn")    # V natural + ones col
        yt_sb = pers.tile([128, 2 * T], BF, tag="yt")  # rows: (h%2); cols: p*T
        rk_sb = pers.tile([128, 2 * NTT], F32, tag="rk")    # K rms recip (even cols)

        nc.vector.memset(vn_sb[:], 1.0)
        biasq_sb = pers.tile([128, 1], F32, tag="biasq")
        nc.vector.memset(biasq_sb[:], 64e-6)
        biask_sb = pers.tile([128, 1], F32, tag="biask")
        nc.vector.memset(biask_sb[:], 1e-6)
        ones128_sb = pers.tile([128, 64], BF, tag="ones128")
        nc.vector.memset(ones128_sb[:], 1.0)

        def emit_dma(ch):
            c0 = ch * TCH
            for kt in range(8):
                nc.sync.dma_start(xt_sb[:, kt * T + c0: kt * T + c0 + TCH],
                                  xt_d[kt][:, c0:c0 + TCH])
            nc.sync.dma_start(cs1_sb[:, c0:c0 + TCH], cs1_d[:, c0:c0 + TCH])
            nc.sync.dma_start(cs2_sb[:, c0:c0 + TCH], cs2_d[:, c0:c0 + TCH])
            for j in range(4):
                tt = ch * 4 + j
                nc.sync.dma_start(ve_sb[:, tt * 64:(tt + 1) * 64], ve_d[tt])

        def rope(ps, nrow, name, csl):
            """RoPE on psum rows [0, nrow); returns f32 SBUF tile ro."""
            A = work.tile([128, TCH], F32, tag="ropeA", name=f"A{name}")
            Bt = work.tile([128, TCH], F32, tag="ropeB", name=f"B{name}")
            ro = work.tile([128, TCH], F32, tag="rope", name=f"ro{name}")
            for r in range(0, nrow, 64):
                h1 = slice(r, r + 32)
                h2 = slice(r + 32, r + 64)
                nc.vector.tensor_mul(A[h1], ps[h1], cs1_sb[h1, csl])
                nc.vector.tensor_mul(Bt[h1], ps[h2], cs1_sb[h2, csl])
                nc.vector.tensor_mul(A[h2], ps[h1], cs2_sb[h1, csl])
                nc.vector.tensor_mul(Bt[h2], ps[h2], cs2_sb[h2, csl])
                nc.vector.tensor_sub(ro[h1], A[h1], Bt[h1])
                nc.vector.tensor_add(ro[h2], A[h2], Bt[h2])
            return ro

        def emit_ph1_a(ch):
            """QKV projection matmuls (PE-dense)."""
            c0 = ch * TCH
            psq = [pbig.tile([128, TCH], F32, tag="mm", name=f"psq{p}_{ch}")
                   for p in range(2)]
            for p in range(2):
                for kt in range(8):
                    nc.tensor.matmul(
                        psq[p][:],
                        wq_sb[:, kt * 256 + p * 128: kt * 256 + (p + 1) * 128],
                        xt_sb[:, kt * T + c0: kt * T + c0 + TCH],
                        start=(kt == 0), stop=(kt == 7))
            pskv = pbig.tile([128, TCH], F32, tag="mm", name=f"pskv{ch}")
            for kt in range(8):
                nc.tensor.matmul(
                    pskv[:], wkv_sb[:, kt * 128:(kt + 1) * 128],
                    xt_sb[:, kt * T + c0: kt * T + c0 + TCH],
                    start=(kt == 0), stop=(kt == 7))
            return psq, pskv

        def emit_ph1_b(ch, psq):
            """Q: rope + rms (ln/exp rsqrt) -> q4 interleaved."""
            c0 = ch * TCH
            csl = slice(c0, c0 + TCH)
            ros = []
            sqs = []
            for p in range(2):
                ro = rope(psq[p], 128, f"q{p}_{ch}", csl)
                sq = work.tile([128, TCH], BF, tag="sq", name=f"sq{p}_{ch}")
                nc.vector.tensor_mul(sq[:], ro[:], ro[:])
                ros.append(ro)
                sqs.append(sq)
            ss = pacc.tile([97, TCH], F32, tag="acc", name=f"ss{ch}")
            nc.tensor.matmul(ss[0:33], selq_sb[:], sqs[0][:],
                             start=True, stop=True)
            nc.tensor.matmul(ss[64:97], selq_sb[:], sqs[1][:],
                             start=True, stop=True)
            lnt = work.tile([97, TCH], F32, tag="lnt", name=f"lnt{ch}")
            nc.scalar.activation(lnt[:], ss[:], AF.Ln, bias=biasq_sb[0:97],
                                 scale=1.0)
            rq = work.tile([97, TCH], BF, tag="rq", name=f"rq{ch}")
            nc.scalar.activation(rq[:], lnt[:], AF.Exp, scale=-0.5)
            for p in range(2):
                bcps = psm.tile([128, TCH], F32, tag="sm", name=f"bcps{p}_{ch}")
                r0, r1 = 64 * p, 64 * p + 32
                nc.tensor.matmul(bcps[0:64], ones128_sb[r0:r0 + 1, :],
                                 rq[r0:r0 + 1, :], start=True, stop=True,
                                 tile_position=(r0, 0))
                nc.tensor.matmul(bcps[64:128], ones128_sb[r1:r1 + 1, :],
                                 rq[r1:r1 + 1, :], start=True, stop=True,
                                 tile_position=(r1, 64))
                ro = ros[p]
                for i in range(2):
                    h = 2 * p + i
                    m = (h % 2) * 2 + h // 2      # column-block order (0,2,1,3)
                    nc.vector.tensor_mul(
                        qn_sb[:, m * T + c0: m * T + c0 + TCH],
                        ro[64 * i: 64 * i + 64, :],
                        bcps[64 * i: 64 * i + 64, :])

        def emit_ph1_c(ch, pskv):
            """K: rope + rms->rk; V: transpose + gated ve add -> vn."""
            c0 = ch * TCH
            csl = slice(c0, c0 + TCH)
            ro = rope(pskv, 64, f"k{ch}", csl)
            nc.vector.tensor_copy(kt_sb[:, c0:c0 + TCH], ro[0:64])
            sqk = work.tile([64, TCH], BF, tag="sqk", name=f"sqk{ch}")
            nc.vector.tensor_mul(sqk[:], ro[0:64], ro[0:64])
            ssk = psm.tile([1, TCH], F32, tag="sm", name=f"ssk{ch}")
            nc.tensor.matmul(ssk[:], on64_sb[:], sqk[:], start=True, stop=True)
            sskb = work.tile([1, TCH], BF, tag="sskb", name=f"sskb{ch}")
            nc.vector.tensor_copy(sskb[:], ssk[:])
            rkT = psm.tile([128, 8], BF, tag="sm", name=f"rkT{ch}")
            for j in range(4):
                nc.tensor.transpose(rkT[:, 2 * j:2 * j + 1],
                                    sskb[:, j * 128:(j + 1) * 128], id1_sb[:])
            lnk = work.tile([128, 8], F32, tag="lnk", name=f"lnk{ch}")
            nc.scalar.activation(lnk[:], rkT[:], AF.Ln, bias=biask_sb[:],
                                 scale=1.0 / 64)
            nc.scalar.activation(rk_sb[:, ch * 8: ch * 8 + 8], lnk[:], AF.Exp,
                                 scale=-0.5)
            # gate: g = 2*sigmoid(z) = 2/(1+exp(-z)), z = x[:, :32] @ wg
            gps = psm.tile([1, TCH], F32, tag="sm", name=f"gps{ch}")
            nc.tensor.matmul(gps[:], wg_sb[:],
                             xt_sb[0:32, c0:c0 + TCH], start=True, stop=True)
            gu = work.tile([1, TCH], F32, tag="gu", name=f"gu{ch}")
            nc.scalar.activation(gu[:], gps[:], AF.Exp, scale=-1.0)
            nc.vector.tensor_scalar_add(gu[:], gu[:], 1.0)
            gr = work.tile([1, TCH], F32, tag="gr", name=f"gr{ch}")
            nc.vector.reciprocal_approx_fast(gr[:], gu[:])
            grb = work.tile([1, TCH], BF, tag="grb", name=f"grb{ch}")
            nc.vector.tensor_copy(grb[:], gr[:])
            gT = psm.tile([128, 8], BF, tag="sm", name=f"gT{ch}")
            for j in range(4):
                nc.tensor.transpose(gT[:, 2 * j:2 * j + 1],
                                    grb[:, j * 128:(j + 1) * 128], id1_sb[:])
            g_sb = work.tile([128, 8], F32, tag="gsb", name=f"gsb{ch}")
            nc.vector.tensor_copy(g_sb[:], gT[:])
            # V natural + gate * ve
            vt_bf = work.tile([64, TCH], BF, tag="vt", name=f"vt{ch}")
            nc.vector.tensor_copy(vt_bf[:], pskv[64:128])
            for j in range(4):
                tt = ch * 4 + j
                vtp = psm.tile([128, 64], BF, tag="sm", name=f"vtp{tt}")
                nc.tensor.transpose(vtp[:], vt_bf[:, j * 128:(j + 1) * 128],
                                    id_sb[:])
                nc.vector.scalar_tensor_tensor(
                    vn_sb[:, tt * 65: tt * 65 + 64],
                    ve_sb[:, tt * 64:(tt + 1) * 64], g_sb[:, 2 * j:2 * j + 1], vtp[:],
                    op0=OP.mult, op1=OP.add)

        def emit_scores(qt):
            """scores + exp + mask + AV for one q-tile (4 heads wide)."""
            lo = max(0, qt - 4)
            yext = pacc.tile([97, 512], F32, tag="acc", name=f"yext{qt}")
            for kt in range(lo, qt + 1):
                stp = pbig.tile([128, 512], F32, tag="mm", name=f"stp{qt}_{kt}")
                qn_ap = qn_sb[:, :].rearrange("p (m t) -> p m t", m=4)[
                    :, :, qt * 128:(qt + 1) * 128]
                nc.tensor.matmul(stp[:], kt_sb[:, kt * 128:(kt + 1) * 128],
                                 qn_ap, start=True, stop=True)
                pt = ptw.tile([128, 512], BF, tag="pt", name=f"pt{qt}_{kt}")
                nc.scalar.activation(pt[:], stp[:], AF.Exp,
                                     scale=rk_sb[:, 2 * kt:2 * kt + 1])
                eng = nc.gpsimd if MASK_ENGINE == "gpsimd" else nc.vector
                if kt == qt:
                    eng.tensor_mul(pt[:], pt[:], mska_sb[:])
                elif kt == qt - 4:
                    eng.tensor_mul(pt[:], pt[:], mskb_sb[:])
                if DEBUG_TAPS and qt == 0:
                    stpc = work.tile([128, 512], F32, tag="stpc", name="stpc")
                    nc.vector.tensor_copy(stpc[:], stp[:])
                    nc.sync.dma_start(stpdbg_d[:], stpc[:])
                    nc.sync.dma_start(ptdbg_d[:], pt[:])
                nc.tensor.matmul(yext[0:65], vn_sb[:, kt * 65: kt * 65 + 65],
                                 pt[:], start=(kt == lo), stop=(kt == qt))
            return yext

        def emit_post(qt, yext):
            """denominator recip + broadcast + yt writes."""
            den = outw.tile([1, 512], F32, tag="den", name=f"den{qt}")
            nc.vector.tensor_copy(den[:], yext[64:65, :])
            rr = outw.tile([1, 512], F32, tag="rr", name=f"rr{qt}")
            nc.vector.reciprocal_approx_fast(rr[:], den[:])
            if DEBUG_TAPS:
                nc.sync.dma_start(rrdbg_d[qt:qt + 1, :], rr[:])
                denc = outw.tile([1, 512], F32, tag="denc", name=f"denc{qt}")
                nc.vector.tensor_copy(denc[:], yext[64:65, :])
                nc.sync.dma_start(dendbg_d[qt:qt + 1, :], denc[:])
            rrb = outw.tile([1, 512], BF, tag="rrb", name=f"rrb{qt}")
            nc.vector.tensor_copy(rrb[:], rr[:])
            bcq = psm.tile([64, 512], F32, tag="sm", name=f"bcq{qt}")
            nc.tensor.matmul(bcq[:], on1x_sb[:], rrb[:], start=True, stop=True)
            bca = outw.tile([64, 512], BF, tag="bca", name=f"bca{qt}")
            nc.vector.tensor_copy(bca[:], bcq[:])
            for i in range(2):
                dst = yt_sb[64 * i:64 * i + 64, :].rearrange(
                    "p (u t) -> p u t", u=2)[:, :, qt * 128:(qt + 1) * 128]
                srcy = yext[0:64, 256 * i:256 * i + 256].rearrange(
                    "p (u c) -> p u c", u=2)
                srcr = bca[:, 256 * i:256 * i + 256].rearrange(
                    "p (u c) -> p u c", u=2)
                nc.vector.tensor_mul(dst, srcy, srcr)

        def emit_proj(qt):
            ops = pprj.tile([128, 1024], F32, tag="prj", name=f"ops{qt}")
            for cc in range(2):
                for p in range(2):
                    nc.tensor.matmul(
                        ops[:, cc * 512:(cc + 1) * 512],
                        yt_sb[:, p * T + qt * 128: p * T + qt * 128 + 128],
                        wp_sb[:, p * 1024 + cc * 512: p * 1024 + cc * 512 + 512],
                        start=(p == 0), stop=(p == 1))
            o_sb = outw.tile([128, 1024], BF, tag="osb", name=f"osb{qt}")
            nc.scalar.copy(o_sb[:, 0:512], ops[:, 0:512])
            nc.vector.tensor_copy(o_sb[:, 512:1024], ops[:, 512:1024])
            nc.sync.dma_start(out_d[qt * 128:(qt + 1) * 128, :], o_sb[:])

        # ---------------- emission schedule ----------------
        emit_dma(0)
        psq, pskv = emit_ph1_a(0)
        emit_ph1_b(0, psq)
        emit_ph1_c(0, pskv)
        for blk in range(4):
            if blk < 3:
                emit_dma(blk + 1)
                psq, pskv = emit_ph1_a(blk + 1)
                emit_ph1_b(blk + 1, psq)
                emit_ph1_c(blk + 1, pskv)
            for j in range(4):
                qt = blk * 4 + j
                yext = emit_scores(qt)
                emit_post(qt, yext)
                if qt > 0:
                    emit_proj(qt - 1)
        emit_proj(NTT - 1)
        if DEBUG_TAPS:
            nc.sync.dma_start(q4dbg_d[:], qn_sb[:])
            nc.sync.dma_start(ktdbg_d[:], kt_sb[:])
            nc.sync.dma_start(vndbg_d[:], vn_sb[:])
            nc.sync.dma_start(rkdbg_d[:], rk_sb[:])
            for p in range(2):
                nc.sync.dma_start(ytdbg_d[p], yt_sb[:, p * T:(p + 1) * T])

    nc.compile()
    return nc


def _prep_inputs(x, ve, cos, sin, Wq, Wk, Wv, Wproj, Wgate):
    """Build the 8 per-core input maps (host-side sharding + layout prep)."""
    cosT = np.ascontiguousarray(cos.T).astype(np.float32)   # [32, T]
    sinT = np.ascontiguousarray(sin.T).astype(np.float32)
    cs1 = np.concatenate([cosT, sinT, cosT, sinT], 0)       # [128, T]
    cs2 = np.concatenate([sinT, cosT, sinT, cosT], 0)
    mska = np.tile(np.triu(np.ones((128, 128), np.float32)), (1, 4)).astype(BF16)
    mskb = np.tile(np.tril(np.ones((128, 128), np.float32)), (1, 4)).astype(BF16)
    ident = np.eye(64, dtype=BF16)
    selq = np.zeros((128, 33), np.float32)
    selq[0:64, 0] = 1.0
    selq[64:128, 32] = 1.0
    selq = selq.astype(BF16)
    ones64 = np.ones((64, 1), BF16)
    ones1x64 = np.ones((1, 64), BF16)
    id1 = np.ones((1, 1), BF16)

    xT = [np.ascontiguousarray(x[b].astype(BF16).T).reshape(8, 128, T)
          for b in range(B)]
    in_maps = []
    for c in range(NCORES):
        b, g = c // 4, c % 4
        wq_g = np.ascontiguousarray(
            Wq[:, g * 256:(g + 1) * 256]).astype(BF16).reshape(8, 128, 256)
        wkv_g = np.concatenate(
            [Wk[:, g * 64:(g + 1) * 64], Wv[:, g * 64:(g + 1) * 64]],
            1).astype(BF16).reshape(8, 128, 128)
        wg_g = np.ascontiguousarray(Wgate[:, g:g + 1]).astype(BF16)
        wp_g = np.ascontiguousarray(
            Wproj[g * 256:(g + 1) * 256, :]).astype(BF16).reshape(2, 128, 1024)
        ve_g = np.ascontiguousarray(
            2.0 * ve[b, :, g * 64:(g + 1) * 64]).astype(BF16).reshape(16, 128, 64)
        in_maps.append({
            "xt": xT[b], "wq": wq_g, "wkv": wkv_g, "wg": wg_g, "wp": wp_g,
            "cs1": cs1, "cs2": cs2, "ve2": ve_g, "mska": mska, "mskb": mskb,
            "ident": ident, "selq": selq, "ones64": ones64,
            "ones1x64": ones1x64, "id1": id1,
        })
    return in_maps


def _run(inputs, trace=False, tmpdir=None):
    if "nc" not in _cache:
        _cache["nc"] = _build()
    nc = _cache["nc"]
    in_maps = _prep_inputs(**inputs)
    res = run_bass_kernel_spmd(nc, in_maps, list(range(NCORES)), trace=trace,
                               tmpdir=tmpdir)
    out = np.zeros((B, T, N_EMBD), np.float32)
    for c in range(NCORES):
        out[c // 4] += np.asarray(res.results[c]["out"], np.float32)
    return out, res


def kernel(**inputs):
    out, _ = _run(inputs)
    return out


# revision 20
# speedup vs baseline: 1.9160x; 1.2696x over previous
"""Trainium2 Bass kernel for sliding-window causal self-attention (GQA + RoPE +
RMS-norm QK + value-embedding gating).

Sharding: 8 cores = 2 (batch) x 4 (KV groups).  Each core handles one batch
element and one KV head (= 4 query heads), computes a partial output through
the row-slice of Wproj for its heads; the host sums the 4 partials per batch.

v2: head-batched phase 2 (one [128,512] score/exp/AV op per (qt,kt) covering
all 4 query heads), single ACT table set (rsqrt/sigmoid rewritten with ln/exp),
approx reciprocals, gpsimd mask muls, interleaved phase emission, bf16 output.
"""

import sys
import os

for _p in ("/root/.axon_site", "/root/.axon_site/_ro/trn_rl_repo",
           "/root/.axon_site/_ro/pypackages", "/opt/trn_rl_repo"):
    if os.path.isdir(_p) and _p not in sys.path:
        sys.path.append(_p)

import numpy as np
import ml_dtypes
from contextlib import ExitStack

import concourse.bass as bass
import concourse.tile as tile
from concourse import bacc, mybir
from concourse.bass_utils import run_bass_kernel_spmd

BF16 = ml_dtypes.bfloat16
N_HEAD, N_KV, HEAD_DIM, WINDOW, N_EMBD = 16, 4, 64, 512, 1024
B, T = 2, 2048
NCORES = 8
TCH = 512               # token chunk for the projection phase
NCH = T // TCH          # 4
NTT = T // 128          # 16 t-tiles
HPK = N_HEAD // N_KV    # 4 query heads per core

F32 = mybir.dt.float32
BF = mybir.dt.bfloat16
AF = mybir.ActivationFunctionType
OP = mybir.AluOpType

# engine choices (tunable)
MASK_ENGINE = "gpsimd"      # "gpsimd" | "vector"
BCA_GPSIMD_PB = False       # broadcast denom recip via gpsimd partition_broadcast
DEBUG_TAPS = False

_cache = {}


def _patch_act_tables():
    """Restrict the ACT table-set chooser to natural_log_exp_and_others so
    Exp and Ln share one table set (avoids per-chunk ACT_TABLE_LOAD swaps)."""
    import concourse.hw_specs as hw_specs
    import concourse.bacc as bacc_mod
    if getattr(hw_specs, "_act_tables_patched", False):
        return
    orig = hw_specs.get_activation_tables
    def patched(arch):
        tabs = dict(orig(arch))
        keep = "natural_log_exp_and_others"
        return {name: (funcs if name == keep else frozenset())
                for name, funcs in tabs.items()}
    hw_specs._act_tables_patched = True
    hw_specs.get_activation_tables = patched
    bacc_mod.get_activation_tables = patched


def _build():
    _patch_act_tables()
    nc = bacc.Bacc("TRN2", target_bir_lowering=False, debug=False,
                   num_devices=NCORES)

    xt_d = nc.dram_tensor("xt", [8, 128, T], BF, kind="ExternalInput")
    wq_d = nc.dram_tensor("wq", [8, 128, 256], BF, kind="ExternalInput")
    wkv_d = nc.dram_tensor("wkv", [8, 128, 128], BF, kind="ExternalInput")
    wg_d = nc.dram_tensor("wg", [32, 1], BF, kind="ExternalInput")
    wp_d = nc.dram_tensor("wp", [2, 128, 1024], BF, kind="ExternalInput")
    cs1_d = nc.dram_tensor("cs1", [128, T], F32, kind="ExternalInput")
    cs2_d = nc.dram_tensor("cs2", [128, T], F32, kind="ExternalInput")
    ve_d = nc.dram_tensor("ve2", [16, 128, 64], BF, kind="ExternalInput")
    mska_d = nc.dram_tensor("mska", [128, 512], BF, kind="ExternalInput")
    mskb_d = nc.dram_tensor("mskb", [128, 512], BF, kind="ExternalInput")
    id_d = nc.dram_tensor("ident", [64, 64], BF, kind="ExternalInput")
    selq_d = nc.dram_tensor("selq", [128, 33], BF, kind="ExternalInput")
    on64_d = nc.dram_tensor("ones64", [64, 1], BF, kind="ExternalInput")
    on1x_d = nc.dram_tensor("ones1x64", [1, 64], BF, kind="ExternalInput")
    id1_d = nc.dram_tensor("id1", [1, 1], BF, kind="ExternalInput")
    out_d = nc.dram_tensor("out", [T, N_EMBD], BF, kind="ExternalOutput")
    if DEBUG_TAPS:
        q4dbg_d = nc.dram_tensor("q4dbg", [64, 4 * T], BF, kind="ExternalOutput")
        ktdbg_d = nc.dram_tensor("ktdbg", [64, T], BF, kind="ExternalOutput")
        vndbg_d = nc.dram_tensor("vndbg", [128, NTT * 65], BF, kind="ExternalOutput")
        rkdbg_d = nc.dram_tensor("rkdbg", [128, 2 * NTT], F32, kind="ExternalOutput")
        ytdbg_d = nc.dram_tensor("ytdbg", [2, 128, T], BF, kind="ExternalOutput")
        rrdbg_d = nc.dram_tensor("rrdbg", [NTT, 512], F32, kind="ExternalOutput")
        dendbg_d = nc.dram_tensor("dendbg", [NTT, 512], F32, kind="ExternalOutput")
        stpdbg_d = nc.dram_tensor("stpdbg", [128, 512], F32, kind="ExternalOutput")
        ptdbg_d = nc.dram_tensor("ptdbg", [128, 512], BF, kind="ExternalOutput")

    with tile.TileContext(nc) as tc, ExitStack() as ctx:
        pers = ctx.enter_context(tc.tile_pool(name="pers", bufs=1))
        work = ctx.enter_context(tc.tile_pool(name="work", bufs=2))
        ptw = ctx.enter_context(tc.tile_pool(name="ptw", bufs=4))
        outw = ctx.enter_context(tc.tile_pool(name="outw", bufs=2))
        # PSUM pools: 2+2+2+2 banks = 16KB
        pbig = ctx.enter_context(tc.tile_pool(name="pbig", bufs=3, space="PSUM"))
        pacc = ctx.enter_context(tc.tile_pool(name="pacc", bufs=2, space="PSUM"))
        psm = ctx.enter_context(tc.tile_pool(name="psm", bufs=2, space="PSUM"))
        pprj = ctx.enter_context(tc.tile_pool(name="pprj", bufs=1, space="PSUM"))

        # ---- persistent SBUF tiles ----
        xt_sb = pers.tile([128, 8 * T], BF, tag="xt")
        wq_sb = pers.tile([128, 8 * 256], BF, tag="wq")
        wkv_sb = pers.tile([128, 8 * 128], BF, tag="wkv")
        wg_sb = pers.tile([32, 1], BF, tag="wg")
        wp_sb = pers.tile([128, 2 * 1024], BF, tag="wp")
        cs1_sb = pers.tile([128, T], F32, tag="cs1")
        cs2_sb = pers.tile([128, T], F32, tag="cs2")
        ve_sb = pers.tile([128, 16 * 64], BF, tag="ve")
        mska_sb = pers.tile([128, 512], BF, tag="mska")
        mskb_sb = pers.tile([128, 512], BF, tag="mskb")
        id_sb = pers.tile([64, 64], BF, tag="ident")
        selq_sb = pers.tile([128, 33], BF, tag="selq")
        on64_sb = pers.tile([64, 1], BF, tag="on64")
        on1x_sb = pers.tile([1, 64], BF, tag="on1x")
        id1_sb = pers.tile([1, 1], BF, tag="id1")

        # weights + small constants first (needed by every chunk)
        for kt in range(8):
            nc.sync.dma_start(wq_sb[:, kt * 256:(kt + 1) * 256], wq_d[kt])
        for kt in range(8):
            nc.sync.dma_start(wkv_sb[:, kt * 128:(kt + 1) * 128], wkv_d[kt])
        nc.sync.dma_start(wg_sb[:], wg_d[:])
        for p in range(2):
            nc.sync.dma_start(wp_sb[:, p * 1024:(p + 1) * 1024], wp_d[p])
        nc.sync.dma_start(mska_sb[:], mska_d[:])
        nc.sync.dma_start(mskb_sb[:], mskb_d[:])
        nc.sync.dma_start(id_sb[:], id_d[:])
        nc.sync.dma_start(selq_sb[:], selq_d[:])
        nc.sync.dma_start(on64_sb[:], on64_d[:])
        nc.sync.dma_start(on1x_sb[:], on1x_d[:])
        nc.sync.dma_start(id1_sb[:], id1_d[:])

        # ---- persistent intermediates ----
        qn_sb = pers.tile([64, 4 * T], BF, tag="qn")   # Q^T head-major (0,2,1,3)
        kt_sb = pers.tile([64, T], BF, tag="kt")            # K^T
        vn_sb = pers.tile([128, NTT * 65], BF, tag="vn")    # V natural + ones col
        yt_sb = pers.tile([128, 2 * T], BF, tag="yt")  # rows: (h%2); cols: p*T
        rk_sb = pers.tile([128, 2 * NTT], F32, tag="rk")    # K rms recip (even cols)

        nc.vector.memset(vn_sb[:], 1.0)
        biasq_sb = pers.tile([128, 1], F32, tag="biasq")
        nc.vector.memset(biasq_sb[:], 64e-6)
        biask_sb = pers.tile([128, 1], F32, tag="biask")
        nc.vector.memset(biask_sb[:], 1e-6)
        ones128_sb = pers.tile([128, 64], BF, tag="ones128")
        nc.vector.memset(ones128_sb[:], 1.0)

        def emit_dma(ch):
            c0 = ch * TCH
            for kt in range(8):
                nc.sync.dma_start(xt_sb[:, kt * T + c0: kt * T + c0 + TCH],
                                  xt_d[kt][:, c0:c0 + TCH])
            nc.sync.dma_start(cs1_sb[:, c0:c0 + TCH], cs1_d[:, c0:c0 + TCH])
            nc.sync.dma_start(cs2_sb[:, c0:c0 + TCH], cs2_d[:, c0:c0 + TCH])
            for j in range(4):
                tt = ch * 4 + j
                nc.sync.dma_start(ve_sb[:, tt * 64:(tt + 1) * 64], ve_d[tt])

        def rope(ps, nrow, name, csl):
            """RoPE on psum rows [0, nrow); returns f32 SBUF tile ro.

            A = ps*cos (full);  Bhat[32k block] = ps[paired block]*csB where
            csB = [+sin, -sin] per half-pair;  ro = A + Bhat (full)."""
            A = work.tile([128, TCH], F32, tag="ropeA", name=f"A{name}")
            Bt = work.tile([128, TCH], F32, tag="ropeB", name=f"B{name}")
            ro = work.tile([128, TCH], F32, tag="rope", name=f"ro{name}")
            nc.vector.tensor_mul(A[0:nrow], ps[0:nrow], cs1_sb[0:nrow, csl])
            for k in range(nrow // 32):
                kk = k ^ 1
                o = slice(32 * k, 32 * k + 32)
                i = slice(32 * kk, 32 * kk + 32)
                nc.vector.tensor_mul(Bt[o], ps[i], cs2_sb[i, csl])
            nc.vector.tensor_add(ro[0:nrow], A[0:nrow], Bt[0:nrow])
            return ro

        def emit_ph1_a(ch):
            """QKV projection matmuls (PE-dense)."""
            c0 = ch * TCH
            psq = [pbig.tile([128, TCH], F32, tag="mm", name=f"psq{p}_{ch}")
                   for p in range(2)]
            for p in range(2):
                for kt in range(8):
                    nc.tensor.matmul(
                        psq[p][:],
                        wq_sb[:, kt * 256 + p * 128: kt * 256 + (p + 1) * 128],
                        xt_sb[:, kt * T + c0: kt * T + c0 + TCH],
                        start=(kt == 0), stop=(kt == 7))
            pskv = pbig.tile([128, TCH], F32, tag="mm", name=f"pskv{ch}")
            for kt in range(8):
                nc.tensor.matmul(
                    pskv[:], wkv_sb[:, kt * 128:(kt + 1) * 128],
                    xt_sb[:, kt * T + c0: kt * T + c0 + TCH],
                    start=(kt == 0), stop=(kt == 7))
            return psq, pskv

        def emit_ph1_b(ch, psq):
            """Q: rope + rms (ln/exp rsqrt) -> q4 interleaved."""
            c0 = ch * TCH
            csl = slice(c0, c0 + TCH)
            ros = []
            sqs = []
            for p in range(2):
                ro = rope(psq[p], 128, f"q{p}_{ch}", csl)
                sq = work.tile([128, TCH], BF, tag="sq", name=f"sq{p}_{ch}")
                nc.vector.tensor_mul(sq[:], ro[:], ro[:])
                ros.append(ro)
                sqs.append(sq)
            ss = pacc.tile([97, TCH], F32, tag="acc", name=f"ss{ch}")
            nc.tensor.matmul(ss[0:33], selq_sb[:], sqs[0][:],
                             start=True, stop=True)
            nc.tensor.matmul(ss[64:97], selq_sb[:], sqs[1][:],
                             start=True, stop=True)
            lnt = work.tile([97, TCH], F32, tag="lnt", name=f"lnt{ch}")
            nc.scalar.activation(lnt[:], ss[:], AF.Ln, bias=biasq_sb[0:97],
                                 scale=1.0)
            rq = work.tile([97, TCH], BF, tag="rq", name=f"rq{ch}")
            nc.scalar.activation(rq[:], lnt[:], AF.Exp, scale=-0.5)
            for p in range(2):
                bcps = psm.tile([128, TCH], F32, tag="sm", name=f"bcps{p}_{ch}")
                r0, r1 = 64 * p, 64 * p + 32
                nc.tensor.matmul(bcps[0:64], ones128_sb[r0:r0 + 1, :],
                                 rq[r0:r0 + 1, :], start=True, stop=True,
                                 tile_position=(r0, 0))
                nc.tensor.matmul(bcps[64:128], ones128_sb[r1:r1 + 1, :],
                                 rq[r1:r1 + 1, :], start=True, stop=True,
                                 tile_position=(r1, 64))
                ro = ros[p]
                for i in range(2):
                    h = 2 * p + i
                    m = (h % 2) * 2 + h // 2      # column-block order (0,2,1,3)
                    nc.vector.tensor_mul(
                        qn_sb[:, m * T + c0: m * T + c0 + TCH],
                        ro[64 * i: 64 * i + 64, :],
                        bcps[64 * i: 64 * i + 64, :])

        def emit_ph1_c(ch, pskv):
            """K: rope + rms->rk; V: transpose + gated ve add -> vn."""
            c0 = ch * TCH
            csl = slice(c0, c0 + TCH)
            ro = rope(pskv, 64, f"k{ch}", csl)
            nc.vector.tensor_copy(kt_sb[:, c0:c0 + TCH], ro[0:64])
            sqk = work.tile([64, TCH], BF, tag="sqk", name=f"sqk{ch}")
            nc.vector.tensor_mul(sqk[:], ro[0:64], ro[0:64])
            ssk = psm.tile([1, TCH], F32, tag="sm", name=f"ssk{ch}")
            nc.tensor.matmul(ssk[:], on64_sb[:], sqk[:], start=True, stop=True)
            sskb = work.tile([1, TCH], BF, tag="sskb", name=f"sskb{ch}")
            nc.vector.tensor_copy(sskb[:], ssk[:])
            rkT = psm.tile([128, 8], BF, tag="sm", name=f"rkT{ch}")
            for j in range(4):
                nc.tensor.transpose(rkT[:, 2 * j:2 * j + 1],
                                    sskb[:, j * 128:(j + 1) * 128], id1_sb[:])
            lnk = work.tile([128, 8], F32, tag="lnk", name=f"lnk{ch}")
            nc.scalar.activation(lnk[:], rkT[:], AF.Ln, bias=biask_sb[:],
                                 scale=1.0 / 64)
            nc.scalar.activation(rk_sb[:, ch * 8: ch * 8 + 8], lnk[:], AF.Exp,
                                 scale=-0.5)
            # gate: g = 2*sigmoid(z) = 2/(1+exp(-z)), z = x[:, :32] @ wg
            gps = psm.tile([1, TCH], F32, tag="sm", name=f"gps{ch}")
            nc.tensor.matmul(gps[:], wg_sb[:],
                             xt_sb[0:32, c0:c0 + TCH], start=True, stop=True)
            gu = work.tile([1, TCH], F32, tag="gu", name=f"gu{ch}")
            nc.scalar.activation(gu[:], gps[:], AF.Exp, scale=-1.0)
            nc.vector.tensor_scalar_add(gu[:], gu[:], 1.0)
            gr = work.tile([1, TCH], F32, tag="gr", name=f"gr{ch}")
            nc.vector.reciprocal_approx_fast(gr[:], gu[:])
            grb = work.tile([1, TCH], BF, tag="grb", name=f"grb{ch}")
            nc.vector.tensor_copy(grb[:], gr[:])
            gT = psm.tile([128, 8], BF, tag="sm", name=f"gT{ch}")
            for j in range(4):
                nc.tensor.transpose(gT[:, 2 * j:2 * j + 1],
                                    grb[:, j * 128:(j + 1) * 128], id1_sb[:])
            g_sb = work.tile([128, 8], F32, tag="gsb", name=f"gsb{ch}")
            nc.vector.tensor_copy(g_sb[:], gT[:])
            # V natural + gate * ve
            vt_bf = work.tile([64, TCH], BF, tag="vt", name=f"vt{ch}")
            nc.vector.tensor_copy(vt_bf[:], pskv[64:128])
            for j in range(4):
                tt = ch * 4 + j
                vtp = psm.tile([128, 64], BF, tag="sm", name=f"vtp{tt}")
                nc.tensor.transpose(vtp[:], vt_bf[:, j * 128:(j + 1) * 128],
                                    id_sb[:])
                nc.vector.scalar_tensor_tensor(
                    vn_sb[:, tt * 65: tt * 65 + 64],
                    ve_sb[:, tt * 64:(tt + 1) * 64], g_sb[:, 2 * j:2 * j + 1], vtp[:],
                    op0=OP.mult, op1=OP.add)

        def emit_scores(qt):
            """scores + exp + mask + AV for one q-tile (4 heads wide)."""
            lo = max(0, qt - 4)
            kts = list(range(lo, qt + 1))
            yext = pacc.tile([97, 512], F32, tag="acc", name=f"yext{qt}")
            qn_ap = qn_sb[:, :].rearrange("p (m t) -> p m t", m=4)[
                :, :, qt * 128:(qt + 1) * 128]

            def s_mm(kt):
                stp = pbig.tile([128, 512], F32, tag="mm", name=f"stp{qt}_{kt}")
                nc.tensor.matmul(stp[:], kt_sb[:, kt * 128:(kt + 1) * 128],
                                 qn_ap, start=True, stop=True)
                return stp

            def pexp(kt, stp):
                pt = ptw.tile([128, 512], BF, tag="pt", name=f"pt{qt}_{kt}")
                nc.scalar.activation(pt[:], stp[:], AF.Exp,
                                     scale=rk_sb[:, 2 * kt:2 * kt + 1])
                eng = nc.gpsimd if MASK_ENGINE == "gpsimd" else nc.vector
                if kt == qt:
                    eng.tensor_mul(pt[:], pt[:], mska_sb[:])
                elif kt == qt - 4:
                    eng.tensor_mul(pt[:], pt[:], mskb_sb[:])
                return pt

            def av_mm(kt, pt):
                nc.tensor.matmul(yext[0:65], vn_sb[:, kt * 65: kt * 65 + 65],
                                 pt[:], start=(kt == lo), stop=(kt == qt))

            stps = {}
            for j, kt in enumerate(kts[:3]):
                stps[kt] = s_mm(kt)
            pts = {}
            for j, kt in enumerate(kts):
                pts[kt] = pexp(kt, stps[kt])
                if j + 3 < len(kts):
                    stps[kts[j + 3]] = s_mm(kts[j + 3])
                av_mm(kt, pts[kt])
            return yext

        def emit_post(qt, yext):
            """denominator recip + broadcast + yt writes."""
            den = outw.tile([1, 512], F32, tag="den", name=f"den{qt}")
            nc.scalar.copy(den[:], yext[64:65, :])
            rr = outw.tile([1, 512], F32, tag="rr", name=f"rr{qt}")
            nc.vector.reciprocal_approx_fast(rr[:], den[:])
            if DEBUG_TAPS:
                nc.sync.dma_start(rrdbg_d[qt:qt + 1, :], rr[:])
                denc = outw.tile([1, 512], F32, tag="denc", name=f"denc{qt}")
                nc.vector.tensor_copy(denc[:], yext[64:65, :])
                nc.sync.dma_start(dendbg_d[qt:qt + 1, :], denc[:])
            rrb = outw.tile([1, 512], BF, tag="rrb", name=f"rrb{qt}")
            nc.vector.tensor_copy(rrb[:], rr[:])
            bcq = psm.tile([64, 512], F32, tag="sm", name=f"bcq{qt}")
            nc.tensor.matmul(bcq[:], on1x_sb[:], rrb[:], start=True, stop=True)
            bca = outw.tile([64, 512], BF, tag="bca", name=f"bca{qt}")
            nc.vector.tensor_copy(bca[:], bcq[:])
            for i in range(2):
                dst = yt_sb[64 * i:64 * i + 64, :].rearrange(
                    "p (u t) -> p u t", u=2)[:, :, qt * 128:(qt + 1) * 128]
                srcy = yext[0:64, 256 * i:256 * i + 256].rearrange(
                    "p (u c) -> p u c", u=2)
                srcr = bca[:, 256 * i:256 * i + 256].rearrange(
                    "p (u c) -> p u c", u=2)
                nc.vector.tensor_mul(dst, srcy, srcr)

        def emit_proj(qt):
            for cc in range(2):
                ops = pprj.tile([128, 512], F32, tag="prj", name=f"ops{qt}_{cc}")
                for p in range(2):
                    nc.tensor.matmul(
                        ops[:],
                        yt_sb[:, p * T + qt * 128: p * T + qt * 128 + 128],
                        wp_sb[:, p * 1024 + cc * 512: p * 1024 + cc * 512 + 512],
                        start=(p == 0), stop=(p == 1))
                o_sb = outw.tile([128, 512], BF, tag="osb", name=f"osb{qt}_{cc}")
                if cc == 0:
                    nc.scalar.copy(o_sb[:], ops[:])
                else:
                    nc.vector.tensor_copy(o_sb[:], ops[:])
                nc.sync.dma_start(
                    out_d[qt * 128:(qt + 1) * 128, cc * 512:(cc + 1) * 512],
                    o_sb[:])

        # ---------------- emission schedule ----------------
        emit_dma(0)
        psq, pskv = emit_ph1_a(0)
        emit_ph1_b(0, psq)
        emit_ph1_c(0, pskv)
        for blk in range(4):
            if blk < 3:
                emit_dma(blk + 1)
                psq, pskv = emit_ph1_a(blk + 1)
                emit_ph1_b(blk + 1, psq)
                emit_ph1_c(blk + 1, pskv)
            for j in range(4):
                qt = blk * 4 + j
                yext = emit_scores(qt)
                emit_post(qt, yext)
                if qt > 0:
                    emit_proj(qt - 1)
        emit_proj(NTT - 1)
        if DEBUG_TAPS:
            nc.sync.dma_start(q4dbg_d[:], qn_sb[:])
            nc.sync.dma_start(ktdbg_d[:], kt_sb[:])
            nc.sync.dma_start(vndbg_d[:], vn_sb[:])
            nc.sync.dma_start(rkdbg_d[:], rk_sb[:])
            for p in range(2):
                nc.sync.dma_start(ytdbg_d[p], yt_sb[:, p * T:(p + 1) * T])

    nc.compile()
    return nc


def _prep_inputs(x, ve, cos, sin, Wq, Wk, Wv, Wproj, Wgate):
    """Build the 8 per-core input maps (host-side sharding + layout prep)."""
    cosT = np.ascontiguousarray(cos.T).astype(np.float32)   # [32, T]
    sinT = np.ascontiguousarray(sin.T).astype(np.float32)
    cs1 = np.concatenate([cosT, cosT, cosT, cosT], 0)       # [128, T] cos
    cs2 = np.concatenate([sinT, -sinT, sinT, -sinT], 0)     # B-mul table
    mska = np.tile(np.triu(np.ones((128, 128), np.float32)), (1, 4)).astype(BF16)
    mskb = np.tile(np.tril(np.ones((128, 128), np.float32)), (1, 4)).astype(BF16)
    ident = np.eye(64, dtype=BF16)
    selq = np.zeros((128, 33), np.float32)
    selq[0:64, 0] = 1.0
    selq[64:128, 32] = 1.0
    selq = selq.astype(BF16)
    ones64 = np.ones((64, 1), BF16)
    ones1x64 = np.ones((1, 64), BF16)
    id1 = np.ones((1, 1), BF16)

    xT = [np.ascontiguousarray(x[b].astype(BF16).T).reshape(8, 128, T)
          for b in range(B)]
    in_maps = []
    for c in range(NCORES):
        b, g = c // 4, c % 4
        wq_g = np.ascontiguousarray(
            Wq[:, g * 256:(g + 1) * 256]).astype(BF16).reshape(8, 128, 256)
        wkv_g = np.concatenate(
            [Wk[:, g * 64:(g + 1) * 64], Wv[:, g * 64:(g + 1) * 64]],
            1).astype(BF16).reshape(8, 128, 128)
        wg_g = np.ascontiguousarray(Wgate[:, g:g + 1]).astype(BF16)
        wp_g = np.ascontiguousarray(
            Wproj[g * 256:(g + 1) * 256, :]).astype(BF16).reshape(2, 128, 1024)
        ve_g = np.ascontiguousarray(
            2.0 * ve[b, :, g * 64:(g + 1) * 64]).astype(BF16).reshape(16, 128, 64)
        in_maps.append({
            "xt": xT[b], "wq": wq_g, "wkv": wkv_g, "wg": wg_g, "wp": wp_g,
            "cs1": cs1, "cs2": cs2, "ve2": ve_g, "mska": mska, "mskb": mskb,
            "ident": ident, "selq": selq, "ones64": ones64,
            "ones1x64": ones1x64, "id1": id1,
        })
    return in_maps


def _run(inputs, trace=False, tmpdir=None):
    if "nc" not in _cache:
        _cache["nc"] = _build()
    nc = _cache["nc"]
    in_maps = _prep_inputs(**inputs)
    res = run_bass_kernel_spmd(nc, in_maps, list(range(NCORES)), trace=trace,
                               tmpdir=tmpdir)
    out = np.zeros((B, T, N_EMBD), np.float32)
    for c in range(NCORES):
        out[c // 4] += np.asarray(res.results[c]["out"], np.float32)
    return out, res


def kernel(**inputs):
    out, _ = _run(inputs)
    return out
